# revision 7
# baseline (speedup 1.0000x reference)
"""MetaPathTransformer Trainium2 kernel (8 NeuronCores, Bass/Tile).

Math: the reference computes heads = inv(D) @ (M0@M1@M2@M3) @ V per
(head, batch), with M_i = sum_a soft[h,a,i] * adjacency[b,a] and D the
(diagonal-by-construction) degree matrix.  We reassociate the chain
right-to-left so every step is [N,N]@[N,256] instead of [N,N]@[N,N],
expand each step as per-relation products accumulated in PSUM and
combined on DVE with per-partition softmax coefficients, and apply
inv(D) as a per-row reciprocal scale.  Matmuls run in float32r
(TF32-like, ~1e-4 rel err, 4x the fp32 PE rate).

Sharding (8 cores): core c -> (b = c>>2, fc = (c>>1)&1, nh = c&1).
Each core holds A^T[b, :, :, nh-slice] (host-pre-transposed, 18.9MB,
SBUF-resident across all 4 chain steps) and runs the chain for its 4
heads (f-chunk fc), producing T^T[fc, nh] pieces; a 2-way AllGather per
step between n-half partners restores full-n T; a final 2-way AllGather
between f-chunk partners assembles attn^T for the core's n-half, and
each core computes the W0/LN/FFN tail for its 512-row half (duplicated
with its fc-partner; host keeps the fc=0 copies).
"""

import sys

try:
    import concourse.bass as bass  # noqa: F401
except ImportError:  # pragma: no cover
    for _p in ("/opt/trn_rl_repo", "/root/.axon_site/_ro/trn_rl_repo"):
        if _p not in sys.path:
            sys.path.insert(0, _p)
    import concourse.bass as bass  # noqa: F401

import numpy as np

import concourse.mybir as mybir
import concourse.tile as tile
from concourse import bacc
from concourse.bass_utils import run_bass_kernel_spmd

B, A, N, P, D, H = 2, 9, 1024, 4, 256, 8
DH = D // H
EPS = 1e-12
NCORES = 8
NH = N // 2          # n-half per core
FC = 128             # f-chunk (4 heads) per core
NQ = 256             # n-quarter per PSUM accumulation group
MC = N // 128        # m-chunks

F32 = mybir.dt.float32
F32R = mybir.dt.float32r
ALU = mybir.AluOpType
ACTF = mybir.ActivationFunctionType

_CACHE: dict = {}


def _build_nc():
    nc = bacc.Bacc("TRN2", target_bir_lowering=False, debug=False, num_devices=NCORES)

    dp = nc.declare_dram_parameter
    at_in = dp("at", [A, MC, 128, NH], F32R, isOutput=False)
    xt_in = dp("xt", [2, 128, N], F32R, isOutput=False)          # x[b]^T, d-chunked
    xtail_in = dp("xtail", [4, 128, D], F32, isOutput=False)      # x rows of our half
    wv_in = dp("wv", [2, 128, FC], F32R, isOutput=False)          # Wv_cat[:, fslice]
    bv_in = dp("bv", [MC, 128, FC], F32, isOutput=False)          # Bv_cat[:, fslice]
    w0_in = dp("w0", [2, 128, D], F32R, isOutput=False)
    w1_in = dp("w1", [2, 128, 2 * D], F32R, isOutput=False)
    w2_in = dp("w2", [4, 128, D], F32R, isOutput=False)
    cv_in = dp("cv", [128, P, A], F32, isOutput=False)            # soft coefs per f-row
    invd_in = dp("invd", [128, NH], F32, isOutput=False)          # inv degree bcast
    g2_in = dp("g2", [128, D], F32, isOutput=False)
    b2_in = dp("b2", [128, D], F32, isOutput=False)
    gf_in = dp("gf", [128, 2 * D], F32, isOutput=False)
    bf_in = dp("bf", [128, 2 * D], F32, isOutput=False)
    b1_in = dp("b1", [128, 2 * D], F32, isOutput=False)
    b2f_in = dp("b2f", [128, D], F32, isOutput=False)
    id_in = dp("ident", [128, 128], F32R, isOutput=False)
    out_p = dp("out", [4, 128, D], F32, isOutput=True)

    with tile.TileContext(nc) as tc:
        with (
            tc.tile_pool(name="atp", bufs=A * MC) as atp,
            tc.tile_pool(name="cst", bufs=1) as cst,
            tc.tile_pool(name="wrk", bufs=1) as wrk,
            tc.tile_pool(name="bvp", bufs=2) as bvp,
            tc.tile_pool(name="tt", bufs=1) as tt,
            tc.tile_pool(name="ps", bufs=4, space="PSUM") as ps,
            tc.tile_pool(name="tp", bufs=2, space="PSUM") as tp,
            tc.tile_pool(name="dram", bufs=1, space="DRAM") as dram,
        ):
            # ---- constants / small inputs ----
            ident = cst.tile([128, 128], F32R)
            nc.sync.dma_start(ident[:], id_in[:])
            cv = cst.tile([128, P, A], F32)
            nc.sync.dma_start(cv[:], cv_in[:])
            wv = cst.tile([128, 2, FC], F32R)
            nc.sync.dma_start(wv[:], wv_in.rearrange("c p f -> p c f"))
            w0 = cst.tile([128, 2, D], F32R)
            nc.sync.dma_start(w0[:], w0_in.rearrange("c p f -> p c f"))
            w1 = cst.tile([128, 2, 2 * D], F32R)
            nc.sync.dma_start(w1[:], w1_in.rearrange("c p f -> p c f"))
            w2 = cst.tile([128, 4, D], F32R)
            nc.sync.dma_start(w2[:], w2_in.rearrange("c p f -> p c f"))
            invd = cst.tile([128, NH], F32)
            nc.sync.dma_start(invd[:], invd_in[:])
            g2b = cst.tile([128, D], F32)
            nc.sync.dma_start(g2b[:], g2_in[:])
            b2b = cst.tile([128, D], F32)
            nc.sync.dma_start(b2b[:], b2_in[:])
            gfb = cst.tile([128, 2 * D], F32)
            nc.sync.dma_start(gfb[:], gf_in[:])
            bfb = cst.tile([128, 2 * D], F32)
            nc.sync.dma_start(bfb[:], bf_in[:])
            b1b = cst.tile([128, 2 * D], F32)
            nc.sync.dma_start(b1b[:], b1_in[:])
            b2fb = cst.tile([128, D], F32)
            nc.sync.dma_start(b2fb[:], b2f_in[:])
            xtl = cst.tile([128, 4, D], F32)
            nc.sync.dma_start(xtl[:], xtail_in.rearrange("c p f -> p c f"))
            epst = cst.tile([128, 1], F32)
            nc.vector.memset(epst[:], EPS)

            # x^T for the V projection
            xt = wrk.tile([128, 2, N], F32R, tag="big8")
            nc.sync.dma_start(xt[:], xt_in.rearrange("c p f -> p c f"))

            # ---- adjacency^T tiles (SBUF-resident all 4 steps) ----
            at = {}
            for a in range(A):
                for m in range(MC):
                    t = atp.tile([128, NH], F32R, tag="AT")
                    nc.sync.dma_start(t[:], at_in[a, m])
                    at[(a, m)] = t

            # ---- V = x @ Wv_cat[:, fslice] + Bv  ->  T0 [m-part, mchunk, f] ----
            tcur = tt.tile([128, MC, FC], F32R, tag="T")
            for m in range(MC):
                pv = ps.tile([128, FC], F32, tag="pa")
                for dc in range(2):
                    nc.tensor.matmul(
                        pv[:],
                        xt[:, dc, m * 128:(m + 1) * 128],
                        wv[:, dc, :],
                        start=(dc == 0),
                        stop=(dc == 1),
                    )
                bvt = bvp.tile([128, FC], F32, tag="bv")
                nc.sync.dma_start(bvt[:], bv_in[m])
                nc.vector.tensor_add(tcur[:, m, :], pv[:], bvt[:])

            # ---- chain: 4 steps of T <- sum_a c[s,a] * (A_a @ T) ----
            ag2_nh = [[2 * g, 2 * g + 1] for g in range(NCORES // 2)]
            ag2_fc = [[base + nh_, base + nh_ + 2]
                      for base in range(0, NCORES, 4) for nh_ in range(2)]

            attn0 = attn1 = None
            for s in range(P):
                acc = wrk.tile([128, NH], F32, tag="f1")
                for nq in range(NH // NQ):
                    nsl = slice(nq * NQ, (nq + 1) * NQ)
                    for a in range(A):
                        pa = ps.tile([128, NQ], F32, tag="pa")
                        for m in range(MC):
                            nc.tensor.matmul(
                                pa[:],
                                tcur[:, m, :],
                                at[(a, m)][:, nsl],
                                start=(m == 0),
                                stop=(m == MC - 1),
                            )
                        if a == 0:
                            nc.vector.tensor_scalar_mul(
                                acc[:, nsl], pa[:], cv[:, s, 0:1]
                            )
                        else:
                            nc.vector.scalar_tensor_tensor(
                                acc[:, nsl],
                                pa[:],
                                cv[:, s, a:a + 1],
                                acc[:, nsl],
                                op0=ALU.mult,
                                op1=ALU.add,
                            )

                if s < P - 1:
                    # exchange n-halves within the pair, rebuild T [m, f]
                    exin = dram.tile([128, NH], F32R, tag=f"exi{s}")
                    exout = dram.tile([2, 128, NH], F32R, tag=f"exo{s}")
                    accr = wrk.tile([128, NH], F32R, tag="accr")
                    nc.vector.tensor_copy(accr[:], acc[:])
                    nc.sync.dma_start(exin[:], accr[:])
                    nc.gpsimd.collective_compute(
                        "AllGather",
                        ALU.bypass,
                        replica_groups=ag2_nh,
                        ins=[exin.opt()],
                        outs=[exout.opt()],
                    )
                    tnt = wrk.tile([128, N], F32R, tag="big8")
                    nc.sync.dma_start(tnt[:, :NH], exout[0])
                    nc.sync.dma_start(tnt[:, NH:], exout[1])
                    tnext = tt.tile([128, MC, FC], F32R, tag="T")
                    for m in range(MC):
                        ptr = tp.tile([128, 128], F32R, tag="tp")
                        nc.tensor.transpose(
                            ptr[:], tnt[:, m * 128:(m + 1) * 128], ident[:]
                        )
                        nc.vector.tensor_copy(tnext[:, m, :], ptr[:])
                    tcur = tnext
                else:
                    # final step: inv(degree) row scale, then gather the
                    # fc-partner's piece -> full attn^T for our n-half
                    piece = wrk.tile([128, NH], F32R, tag="accr")
                    nc.vector.tensor_mul(piece[:], acc[:], invd[:])
                    agin = dram.tile([128, NH], F32R, tag="agi")
                    agout = dram.tile([2, 128, NH], F32R, tag="ago")
                    nc.sync.dma_start(agin[:], piece[:])
                    nc.gpsimd.collective_compute(
                        "AllGather",
                        ALU.bypass,
                        replica_groups=ag2_fc,
                        ins=[agin.opt()],
                        outs=[agout.opt()],
                    )
                    attn0 = wrk.tile([128, NH], F32R, tag="at0")
                    attn1 = wrk.tile([128, NH], F32R, tag="at1")
                    nc.sync.dma_start(attn0[:], agout[0])
                    nc.sync.dma_start(attn1[:], agout[1])
            assert attn0 is not None and attn1 is not None

            # ---- tail for our 512-row n-half (4 chunks of 128 rows) ----
            resid = wrk.tile([128, 4, D], F32, tag="resid")
            for i in range(4):
                pr = ps.tile([128, D], F32, tag="pa")
                csl = slice(i * 128, (i + 1) * 128)
                nc.tensor.matmul(pr[:], attn0[:, csl], w0[:, 0, :],
                                 start=True, stop=False)
                nc.tensor.matmul(pr[:], attn1[:, csl], w0[:, 1, :],
                                 start=False, stop=True)
                nc.vector.tensor_add(resid[:, i, :], pr[:], xtl[:, i, :])

                # h = LayerNorm(resid) * gamma2 + beta2
                st = wrk.tile([128, 6], F32, tag="st")
                mv = wrk.tile([128, 2], F32, tag="mv")
                nc.vector.bn_stats(st[:], resid[:, i, :])
                nc.vector.bn_aggr(mv[:], st[:])
                rstd = wrk.tile([128, 1], F32, tag="rstd")
                nc.scalar.activation(rstd[:], mv[:, 1:2], ACTF.Sqrt,
                                     bias=epst[:], scale=1.0)
                nc.vector.reciprocal(rstd[:], rstd[:])
                hn = wrk.tile([128, D], F32, tag="hn")
                nc.vector.tensor_scalar(hn[:], resid[:, i, :], mv[:, 0:1],
                                        rstd[:], op0=ALU.subtract, op1=ALU.mult)
                nc.vector.tensor_mul(hn[:], hn[:], g2b[:])
                hb = wrk.tile([128, D], F32R, tag="hb")
                nc.vector.tensor_add(hb[:], hn[:], b2b[:])

                # h^T for the W1 matmul
                ht = wrk.tile([128, 2, 128], F32R, tag="ht")
                for dc in range(2):
                    ptr = tp.tile([128, 128], F32R, tag="tp")
                    nc.tensor.transpose(ptr[:], hb[:, dc * 128:(dc + 1) * 128],
                                        ident[:])
                    nc.vector.tensor_copy(ht[:, dc, :], ptr[:])

                # f = gelu(h @ W1 + b1), then LayerNorm * gf + bf
                pf = ps.tile([128, 2 * D], F32, tag="pa")
                for dc in range(2):
                    nc.tensor.matmul(pf[:], ht[:, dc, :], w1[:, dc, :],
                                     start=(dc == 0), stop=(dc == 1))
                f1 = wrk.tile([128, 2 * D], F32, tag="f1")
                nc.vector.tensor_add(f1[:], pf[:], b1b[:])
                g1 = wrk.tile([128, 2 * D], F32, tag="g1")
                nc.scalar.activation(g1[:], f1[:], ACTF.Gelu)

                st2 = wrk.tile([128, 6], F32, tag="st")
                mv2 = wrk.tile([128, 2], F32, tag="mv")
                nc.vector.bn_stats(st2[:], g1[:])
                nc.vector.bn_aggr(mv2[:], st2[:])
                rstd2 = wrk.tile([128, 1], F32, tag="rstd")
                nc.scalar.activation(rstd2[:], mv2[:, 1:2], ACTF.Sqrt,
                                     bias=epst[:], scale=1.0)
                nc.vector.reciprocal(rstd2[:], rstd2[:])
                fn = wrk.tile([128, 2 * D], F32, tag="fn")
                nc.vector.tensor_scalar(fn[:], g1[:], mv2[:, 0:1], rstd2[:],
                                        op0=ALU.subtract, op1=ALU.mult)
                nc.vector.tensor_mul(fn[:], fn[:], gfb[:])
                f2 = wrk.tile([128, 2 * D], F32R, tag="f2")
                nc.vector.tensor_add(f2[:], fn[:], bfb[:])

                # f2^T, then out = f2 @ W2 + b2f + resid
                f2t = wrk.tile([128, 4, 128], F32R, tag="f2t")
                for k in range(4):
                    ptr = tp.tile([128, 128], F32R, tag="tp")
                    nc.tensor.transpose(ptr[:], f2[:, k * 128:(k + 1) * 128],
                                        ident[:])
                    nc.vector.tensor_copy(f2t[:, k, :], ptr[:])

                po = ps.tile([128, D], F32, tag="pa")
                for k in range(4):
                    nc.tensor.matmul(po[:], f2t[:, k, :], w2[:, k, :],
                                     start=(k == 0), stop=(k == 3))
                ot = wrk.tile([128, D], F32, tag="ot")
                nc.vector.tensor_add(ot[:], po[:], b2fb[:])
                nc.vector.tensor_add(ot[:], ot[:], resid[:, i, :])
                nc.sync.dma_start(out_p[i], ot[:])

    nc.finalize()
    return nc


def _softmax_relu(kernels):
    r = np.maximum(kernels, 0.0)
    e = np.exp(r - r.max(axis=1, keepdims=True))
    return (e / e.sum(axis=1, keepdims=True)).astype(np.float32)  # [H, A, P]


def _prep_in_maps(adjacency, degree, x, kernels, Wv, Bv, W0, gamma2, beta2,
                  W1, b1, gf, bf, W2, b2f):
    soft = _softmax_relu(np.asarray(kernels, np.float32))
    wv_cat = np.ascontiguousarray(
        np.transpose(np.asarray(Wv, np.float32), (1, 0, 2)).reshape(D, D))
    bv_cat = np.ascontiguousarray(
        np.transpose(np.asarray(Bv, np.float32), (1, 0, 2)).reshape(N, D))
    invd_full = 1.0 / np.diagonal(np.asarray(degree, np.float32),
                                  axis1=1, axis2=2)  # [B, N]
    eye = np.eye(128, dtype=np.float32)
    ones128 = np.ones((128, 1), np.float32)

    g2 = (ones128 * np.asarray(gamma2, np.float32)[None, :])
    b2 = (ones128 * np.asarray(beta2, np.float32)[None, :])
    gfB = (ones128 * np.asarray(gf, np.float32)[None, :])
    bfB = (ones128 * np.asarray(bf, np.float32)[None, :])
    b1B = (ones128 * np.asarray(b1, np.float32)[None, :])
    b2fB = (ones128 * np.asarray(b2f, np.float32)[None, :])
    w0r = np.asarray(W0, np.float32).reshape(2, 128, D)
    w1r = np.asarray(W1, np.float32).reshape(2, 128, 2 * D)
    w2r = np.asarray(W2, np.float32).reshape(4, 128, D)

    adjacency = np.asarray(adjacency, np.float32)
    x = np.asarray(x, np.float32)

    in_maps = []
    for c in range(NCORES):
        b = c >> 2
        fc = (c >> 1) & 1
        nh = c & 1
        nsl = slice(nh * NH, (nh + 1) * NH)
        fsl = slice(fc * FC, (fc + 1) * FC)

        at_c = np.ascontiguousarray(
            adjacency[b].transpose(0, 2, 1)[:, :, nsl]
        ).reshape(A, MC, 128, NH)
        xt_c = np.ascontiguousarray(x[b].T).reshape(2, 128, N)
        xtail_c = np.ascontiguousarray(x[b, nsl]).reshape(4, 128, D)
        # per-f-row softmax coefficients; chain applies step s = mix P-1-s
        fidx = np.arange(fc * FC, (fc + 1) * FC)
        hidx = fidx // DH
        cvec = np.empty((128, P, A), np.float32)
        for s in range(P):
            cvec[:, s, :] = soft[hidx, :, P - 1 - s]
        invd_b = np.ascontiguousarray(
            ones128 * invd_full[b][None, nsl]).astype(np.float32)

        in_maps.append({
            "at": at_c,
            "xt": xt_c,
            "xtail": xtail_c,
            "wv": np.ascontiguousarray(wv_cat[:, fsl]).reshape(2, 128, FC),
            "bv": np.ascontiguousarray(bv_cat[:, fsl]).reshape(MC, 128, FC),
            "w0": w0r, "w1": w1r, "w2": w2r,
            "cv": cvec,
            "invd": invd_b,
            "g2": g2, "b2": b2, "gf": gfB, "bf": bfB, "b1": b1B, "b2f": b2fB,
            "ident": eye,
        })
    return in_maps


def kernel(**inputs) -> np.ndarray:
    if "nc" not in _CACHE:
        _CACHE["nc"] = _build_nc()
    nc = _CACHE["nc"]
    in_maps = _prep_in_maps(**inputs)
    res = run_bass_kernel_spmd(nc, in_maps, core_ids=list(range(NCORES)))
    out = np.empty((B, N, D), np.float32)
    for c in range(NCORES):
        if (c >> 1) & 1:          # fc=1 cores duplicate the fc=0 tails
            continue
        b = c >> 2
        nh = c & 1
        out[b, nh * NH:(nh + 1) * NH] = res.results[c]["out"].reshape(NH, D)
    return out
